# revision 1
# baseline (speedup 1.0000x reference)
import sys

import numpy as np

try:
    import concourse.bass as bass
except ImportError:
    sys.path.insert(0, "/opt/trn_rl_repo")
    import concourse.bass as bass

import concourse.bacc as bacc
import concourse.mybir as mybir
import concourse.tile as tile
from concourse.bass_utils import run_bass_kernel_spmd

F32 = mybir.dt.float32
F32R = mybir.dt.float32r
BF16 = mybir.dt.bfloat16
B, S, D = 2, 2048, 1024
NH, DH = 16, 64
HPC = 4            # heads per core
HF = HPC * DH      # 256 per-core head features
TQ = S // 4        # 512: t-chunk size
QB = 128           # query block (per-core output rows per chunk)
SCALE = 1.0 / float(np.sqrt(DH))

_CACHE = {}


def _build_graph(variant="full"):
    nc = bacc.Bacc(num_devices=8)

    xqT = nc.dram_tensor("xqT", [D, S], BF16, kind="ExternalInput")
    xkT = nc.dram_tensor("xkT", [D, S], BF16, kind="ExternalInput")
    xvT = nc.dram_tensor("xvT", [D, S], BF16, kind="ExternalInput")
    wqT = nc.dram_tensor("wqT", [D, HF], BF16, kind="ExternalInput")
    wkT = nc.dram_tensor("wkT", [D, HF], BF16, kind="ExternalInput")
    wvT = nc.dram_tensor("wvT", [D, HF], BF16, kind="ExternalInput")
    woT = nc.dram_tensor("woT", [D, D], BF16, kind="ExternalInput")
    # lower-triangular ones (incl. diagonal) in [key, query] orientation
    tri = nc.dram_tensor("tri", [128, 128], BF16, kind="ExternalInput")
    # sel[0, r] == 1 iff this core's group rank is r (used to predicate the
    # post-AllGather staging DMA for this core's query block)
    sel = nc.dram_tensor("sel", [1, 4], mybir.dt.uint32, kind="ExternalInput")
    out_q = nc.dram_tensor("out_q", [4 * QB, D], F32, kind="ExternalOutput")

    Exp = mybir.ActivationFunctionType.Exp

    with tile.TileContext(nc) as tc:
        sel_regs = []
        for j in range(4):
            r = nc.sync.alloc_register(f"selreg{j}")
            nc.sync.reg_load(r, sel[0:1, j:j + 1])
            sel_regs.append(nc.sync.snap(r, donate=True, min_val=0, max_val=1))
        with (
            tc.tile_pool(name="dram", bufs=1, space="DRAM") as dramp,
            tc.tile_pool(name="const", bufs=1) as constp,
            tc.tile_pool(name="persist", bufs=1) as pers,
            tc.tile_pool(name="weights", bufs=1) as wpool,
            tc.tile_pool(name="xstage", bufs=2) as xpool,
            tc.tile_pool(name="attn", bufs=5) as apool,
            tc.tile_pool(name="ctx", bufs=3) as cpool,
            tc.tile_pool(name="rb", bufs=2) as rbpool,
            tc.tile_pool(name="rv", bufs=2) as rvpool,
            tc.tile_pool(name="cst", bufs=2) as cstp,
            tc.tile_pool(name="obuf", bufs=2) as obp,
            tc.tile_pool(name="ps_mm", bufs=1, space="PSUM") as ps_mm,
            tc.tile_pool(name="ps_s", bufs=2, space="PSUM") as ps_s,
            tc.tile_pool(name="ps_ctx", bufs=2, space="PSUM") as ps_ctx,
        ):
            # per-chunk collective staging: ccin = this core's ctx features
            # (local rows 0..255) for the whole chunk; agout = concat over the
            # 4-core group = all 1024 global features for the chunk.
            ccin = [dramp.tile([HF, TQ], BF16, name=f"ccin{j}") for j in range(4)]
            agout = [
                dramp.tile([4 * HF, TQ], BF16, name=f"agout{j}") for j in range(4)
            ]

            tri_sb = constp.tile([128, 128], BF16, name="tri_sb")
            nc.sync.dma_start(tri_sb[:], tri[:, :])
            ones_f = constp.tile([1, DH], F32, name="ones_f")
            nc.vector.memset(ones_f[:], 1.0)
            ones_sb = constp.tile([1, DH], F32R, name="ones_sb")
            with nc.allow_low_precision(reason="f32r ones constant"):
                nc.vector.tensor_copy(ones_sb[:], ones_f[:])

            wq_sb = wpool.tile([128, 8, HF], BF16, name="wq_sb")
            wk_sb = wpool.tile([128, 8, HF], BF16, name="wk_sb")
            wv_sb = wpool.tile([128, 8, HF], BF16, name="wv_sb")
            nc.sync.dma_start(wq_sb[:], wqT[:, :].rearrange("(n p) o -> p n o", p=128))
            nc.sync.dma_start(wk_sb[:], wkT[:, :].rearrange("(n p) o -> p n o", p=128))
            nc.sync.dma_start(wv_sb[:], wvT[:, :].rearrange("(n p) o -> p n o", p=128))
            wo_sb = wpool.tile([128, 8, D], BF16, name="wo_sb")
            nc.sync.dma_start(wo_sb[:], woT[:, :].rearrange("(n p) d -> p n d", p=128))

            # Persistent Q^T/K^T in bf16: tile u holds heads (2u, 2u+1)
            # stacked on partitions (64 each). V natural orientation with a
            # ones column (row 64 of the AV product = softmax denominator).
            QT = [pers.tile([128, S], BF16, name=f"QT{u}") for u in range(2)]
            KT = [pers.tile([128, S], BF16, name=f"KT{u}") for u in range(2)]
            NJT = S // 128
            Vb = pers.tile([128, NJT * HPC, DH + 1], BF16, name="Vb")
            nc.vector.memset(Vb[:, :, DH], 1.0)

            for tcc in range(4):
                xq_sb = xpool.tile([128, 8, TQ], BF16, name="xq_st")
                xk_sb = xpool.tile([128, 8, TQ], BF16, name="xk_st")
                xv_sb = xpool.tile([128, 8, TQ], BF16, name="xv_st")
                for xs, src in ((xq_sb, xqT), (xk_sb, xkT), (xv_sb, xvT)):
                    nc.sync.dma_start(
                        xs[:],
                        src[:, bass.ts(tcc, TQ)].rearrange("(n p) t -> p n t", p=128),
                    )

                # Q/K projections for this t-chunk: psum[o128, t512], 8 d-tiles
                for xs, w_sb, dst in ((xq_sb, wq_sb, QT), (xk_sb, wk_sb, KT)):
                    for u in range(2):
                        ps = ps_mm.tile([128, TQ], F32, name="ps")
                        for kd in range(8):
                            nc.tensor.matmul(
                                ps[:],
                                w_sb[:, kd, bass.ts(u, 128)],
                                xs[:, kd, :],
                                start=(kd == 0),
                                stop=(kd == 7),
                            )
                        nc.vector.tensor_copy(dst[u][:, bass.ts(tcc, TQ)], ps[:])

                # V projection: natural orientation [t128, feat256] per j-tile
                for jl in range(4):
                    jt = tcc * 4 + jl
                    psv = ps_mm.tile([128, TQ], F32, name="ps")
                    for kd in range(8):
                        nc.tensor.matmul(
                            psv[:, 0:HF],
                            xv_sb[:, kd, bass.ts(jl, 128)],
                            wv_sb[:, kd, :],
                            start=(kd == 0),
                            stop=(kd == 7),
                        )
                    nc.vector.tensor_copy(
                        Vb[:, jt * HPC:(jt + 1) * HPC, 0:DH],
                        psv[:, 0:HF].rearrange("p (h k) -> p h k", k=DH),
                    )

                # Attention for i-chunk ic == tcc, head pairs u=(2u, 2u+1).
                # Scores for both heads of a pair go into one 2-bank PSUM
                # tile (concurrent row-group matmuls, K=64 each), one merged
                # exp per j-tile. Causality handled by free-dim trimming on
                # the diagonal chunk + a 0/1 triangle multiply on the
                # diagonal 128-block (no -1e9 mask add needed).
                ic = tcc
                n_jt = 4 * ic + 4
                skew = 2
                for u in range(2):
                    pctx = [
                        ps_ctx.tile([DH + 1, TQ], F32, name="pctx") for _ in range(2)
                    ]
                    ats = []
                    lows = []
                    for jt in range(n_jt):
                        p = jt - 4 * ic
                        lo = max(p, 0) * 128
                        ps2 = ps_s.tile([128, 2, TQ], F32, name="ps_sc")
                        for h in range(2):
                            nc.tensor.matmul(
                                ps2[:, h, lo:TQ],
                                KT[u][h * DH:(h + 1) * DH, bass.ts(jt, 128)],
                                QT[u][h * DH:(h + 1) * DH, ic * TQ + lo:(ic + 1) * TQ],
                                start=True,
                                stop=True,
                            )
                        at2 = apool.tile([128, 2, TQ], BF16, name="at2")
                        nc.scalar.activation(
                            at2[:, :, lo:TQ], ps2[:, :, lo:TQ], Exp, scale=SCALE
                        )
                        if p >= 0:
                            for h in range(2):
                                nc.vector.tensor_mul(
                                    at2[:, h, lo:lo + 128],
                                    at2[:, h, lo:lo + 128],
                                    tri_sb[:],
                                )
                        ats.append(at2)
                        lows.append(lo)
                        # AV accumulation skewed behind scores for PE/ACT
                        # pipelining
                        if jt >= skew:
                            pv = jt - skew
                            for h in range(2):
                                nc.tensor.matmul(
                                    pctx[h][:, lows[pv]:TQ],
                                    Vb[:, pv * HPC + 2 * u + h, :],
                                    ats[pv][:, h, lows[pv]:TQ],
                                    start=(pv == 0),
                                    stop=False,
                                )
                    for pv in range(max(n_jt - skew, 0), n_jt):
                        for h in range(2):
                            nc.tensor.matmul(
                                pctx[h][:, lows[pv]:TQ],
                                Vb[:, pv * HPC + 2 * u + h, :],
                                ats[pv][:, h, lows[pv]:TQ],
                                start=(pv == 0),
                                stop=(pv == n_jt - 1),
                            )

                    # Normalize: row DH of pctx is the denominator. Broadcast
                    # 1/denom across 64 partitions via a rank-1 f32r matmul.
                    for h in range(2):
                        gh = 2 * u + h
                        rv = rvpool.tile([1, TQ], F32R, name="rvec")
                        with nc.allow_low_precision(reason="f32r 1/denom bcast"):
                            nc.vector.reciprocal(rv[:], pctx[h][DH:DH + 1, :])
                        pb = ps_mm.tile([DH, TQ], F32, name="pb", bufs=1)
                        nc.tensor.matmul(
                            pb[:],
                            ones_sb[:],
                            rv[:],
                            start=True,
                            stop=True,
                        )
                        rb = rbpool.tile([DH, TQ], F32, name="rbt")
                        nc.scalar.copy(rb[:], pb[:])
                        ctxT = cpool.tile([DH, TQ], BF16, name="ctxT")
                        nc.vector.tensor_mul(ctxT[:], pctx[h][0:DH, :], rb[:])
                        nc.sync.dma_start(
                            ccin[ic][gh * DH:(gh + 1) * DH, :], ctxT[:]
                        )

                nc.gpsimd.collective_compute(
                    "AllGather",
                    mybir.AluOpType.bypass,
                    replica_groups=[[0, 1, 2, 3], [4, 5, 6, 7]],
                    ins=[ccin[ic].opt()],
                    outs=[agout[ic].opt()],
                )

                # Output projection for this core's query block of chunk ic:
                # stage the [1024 feats, 128 q] slice for group rank r via
                # predicated DMAs (exactly one fires at runtime).
                cst = cstp.tile([128, 8, QB], BF16, name="cst")
                for r in range(4):
                    nc.sync.dma_start(
                        cst[:],
                        agout[ic][:, bass.ts(r, QB)].rearrange(
                            "(n p) q -> p n q", p=128
                        ),
                        cond=sel_regs[r],
                    )
                for dc in range(2):
                    pso = ps_mm.tile([128, TQ], F32, name="ps")
                    for kt in range(8):
                        nc.tensor.matmul(
                            pso[0:QB, :],
                            cst[:, kt, :],
                            wo_sb[:, kt, bass.ts(dc, TQ)],
                            start=(kt == 0),
                            stop=(kt == 7),
                        )
                    ob = obp.tile([QB, TQ], F32, name="ob")
                    nc.vector.tensor_copy(ob[:], pso[0:QB, :])
                    nc.sync.dma_start(
                        out_q[bass.ts(ic, QB), bass.ts(dc, TQ)], ob[:]
                    )

    nc.finalize()
    return nc


def _make_in_maps(inputs):
    import ml_dtypes

    bf16 = ml_dtypes.bfloat16
    query, key, value = inputs["query"], inputs["key"], inputs["value"]
    Wq, Wk, Wv, Wo = inputs["Wq"], inputs["Wk"], inputs["Wv"], inputs["Wo"]

    tri_blk = np.tril(np.ones((128, 128), np.float32)).T.astype(bf16)
    woT_full = np.ascontiguousarray(np.asarray(Wo, np.float32).T).astype(bf16)

    xT = {}
    for b in range(2):
        xT[("q", b)] = np.ascontiguousarray(np.asarray(query[b], np.float32).T).astype(bf16)
        xT[("k", b)] = np.ascontiguousarray(np.asarray(key[b], np.float32).T).astype(bf16)
        xT[("v", b)] = np.ascontiguousarray(np.asarray(value[b], np.float32).T).astype(bf16)

    in_maps = []
    for c in range(8):
        b, r = divmod(c, 4)
        rs = slice(r * HF, (r + 1) * HF)
        in_maps.append(
            {
                "xqT": xT[("q", b)],
                "xkT": xT[("k", b)],
                "xvT": xT[("v", b)],
                "wqT": np.ascontiguousarray(np.asarray(Wq[rs], np.float32).T).astype(bf16),
                "wkT": np.ascontiguousarray(np.asarray(Wk[rs], np.float32).T).astype(bf16),
                "wvT": np.ascontiguousarray(np.asarray(Wv[rs], np.float32).T).astype(bf16),
                "woT": woT_full,
                "tri": tri_blk,
                "sel": (np.arange(4, dtype=np.uint32) == r).astype(np.uint32)[None, :],
            }
        )
    return in_maps


def _run(inputs, trace=False):
    if "nc" not in _CACHE:
        _CACHE["nc"] = _build_graph()
    nc = _CACHE["nc"]
    in_maps = _make_in_maps(inputs)
    res = run_bass_kernel_spmd(nc, in_maps, core_ids=list(range(8)), trace=trace)

    out = np.empty((B, S, D), np.float32)
    for c in range(8):
        b, r = divmod(c, 4)
        oq = np.asarray(res.results[c]["out_q"])
        for ic in range(4):
            out[b, ic * TQ + r * QB:ic * TQ + (r + 1) * QB, :] = oq[
                ic * QB:(ic + 1) * QB, :
            ]
    return out, res


def kernel(**inputs):
    out, _ = _run(inputs, trace=False)
    return out



# revision 3
# speedup vs baseline: 1.5820x; 1.5820x over previous
import sys

import numpy as np

try:
    import concourse.bass as bass
except ImportError:
    sys.path.insert(0, "/opt/trn_rl_repo")
    import concourse.bass as bass

import concourse.bacc as bacc
import concourse.mybir as mybir
import concourse.tile as tile
from concourse.bass_utils import run_bass_kernel_spmd

F32 = mybir.dt.float32
F32R = mybir.dt.float32r
BF16 = mybir.dt.bfloat16
B, S, D = 2, 2048, 1024
NH, DH = 16, 64
HPC = 4            # heads per core
HF = HPC * DH      # 256 per-core head features
TQ = S // 4        # 512: t-chunk size
QB = 128           # query block (per-core output rows per chunk)
SCALE = 1.0 / float(np.sqrt(DH))

_CACHE = {}


def _build_graph(variant="full"):
    sim = variant == "sim"
    nc = bacc.Bacc(num_devices=8)

    xqT = nc.dram_tensor("xqT", [D, S], BF16, kind="ExternalInput")
    xkT = nc.dram_tensor("xkT", [D, S], BF16, kind="ExternalInput")
    xvT = nc.dram_tensor("xvT", [D, S], BF16, kind="ExternalInput")
    wqT = nc.dram_tensor("wqT", [D, HF], BF16, kind="ExternalInput")
    wkT = nc.dram_tensor("wkT", [D, HF], BF16, kind="ExternalInput")
    wvT = nc.dram_tensor("wvT", [D, HF], BF16, kind="ExternalInput")
    woT = nc.dram_tensor("woT", [D, D], BF16, kind="ExternalInput")
    # lower-triangular ones (incl. diagonal) in [key, query] orientation
    tri = nc.dram_tensor("tri", [128, 128], BF16, kind="ExternalInput")
    # sel[0, r] == 1 iff this core's group rank is r (used to predicate the
    # post-AllGather staging DMA for this core's query block)
    sel = nc.dram_tensor("sel", [1, 4], mybir.dt.uint32, kind="ExternalInput")
    out_q = nc.dram_tensor("out_q", [4 * QB, D], F32, kind="ExternalOutput")

    Exp = mybir.ActivationFunctionType.Exp

    with tile.TileContext(nc) as tc:
        sel_regs = []
        for j in range(4):
            r = nc.sync.alloc_register(f"selreg{j}")
            nc.sync.reg_load(r, sel[0:1, j:j + 1])
            sel_regs.append(nc.sync.snap(r, donate=True, min_val=0, max_val=1))
        with (
            tc.tile_pool(name="dram", bufs=1, space="DRAM") as dramp,
            tc.tile_pool(name="const", bufs=1) as constp,
            tc.tile_pool(name="persist", bufs=1) as pers,
            tc.tile_pool(name="weights", bufs=1) as wpool,
            tc.tile_pool(name="xstage", bufs=2) as xpool,
            tc.tile_pool(name="attn", bufs=5) as apool,
            tc.tile_pool(name="ctx", bufs=3) as cpool,
            tc.tile_pool(name="rb", bufs=2) as rbpool,
            tc.tile_pool(name="rv", bufs=2) as rvpool,
            tc.tile_pool(name="cst", bufs=2) as cstp,
            tc.tile_pool(name="obuf", bufs=2) as obp,
            tc.tile_pool(name="ps_mm", bufs=1, space="PSUM") as ps_mm,
            tc.tile_pool(name="ps_s", bufs=2, space="PSUM") as ps_s,
            tc.tile_pool(name="ps_ctx", bufs=2, space="PSUM") as ps_ctx,
        ):
            # per-chunk collective staging: ccin = this core's ctx features
            # (local rows 0..255) for the whole chunk; agout = concat over the
            # 4-core group = all 1024 global features for the chunk.
            ccin = [dramp.tile([HF, TQ], BF16, name=f"ccin{j}") for j in range(4)]
            agout = [
                dramp.tile([4 * HF, TQ], BF16, name=f"agout{j}") for j in range(4)
            ]

            tri_sb = constp.tile([128, 128], BF16, name="tri_sb")
            nc.sync.dma_start(tri_sb[:], tri[:, :])
            ones_f = constp.tile([1, DH], F32, name="ones_f")
            nc.vector.memset(ones_f[:], 1.0)
            ones_sb = constp.tile([1, DH], F32R, name="ones_sb")
            with nc.allow_low_precision(reason="f32r ones constant"):
                nc.vector.tensor_copy(ones_sb[:], ones_f[:])

            wq_sb = wpool.tile([128, 8, HF], BF16, name="wq_sb")
            wk_sb = wpool.tile([128, 8, HF], BF16, name="wk_sb")
            wv_sb = wpool.tile([128, 8, HF], BF16, name="wv_sb")
            nc.sync.dma_start(wq_sb[:], wqT[:, :].rearrange("(n p) o -> p n o", p=128))
            nc.sync.dma_start(wk_sb[:], wkT[:, :].rearrange("(n p) o -> p n o", p=128))
            nc.sync.dma_start(wv_sb[:], wvT[:, :].rearrange("(n p) o -> p n o", p=128))
            wo_sb = wpool.tile([128, 8, D], BF16, name="wo_sb")
            nc.sync.dma_start(wo_sb[:], woT[:, :].rearrange("(n p) d -> p n d", p=128))

            # Persistent Q^T/K^T in bf16: tile u holds heads (2u, 2u+1)
            # stacked on partitions (64 each). V natural orientation with a
            # ones column (row 64 of the AV product = softmax denominator).
            QT = [pers.tile([128, S], BF16, name=f"QT{u}") for u in range(2)]
            KT = [pers.tile([128, S], BF16, name=f"KT{u}") for u in range(2)]
            NJT = S // 128
            Vb = pers.tile([128, NJT * HPC, DH + 1], BF16, name="Vb")
            nc.vector.memset(Vb[:, :, DH], 1.0)

            for tcc in range(4):
                xq_sb = xpool.tile([128, 8, TQ], BF16, name="xq_st")
                xk_sb = xpool.tile([128, 8, TQ], BF16, name="xk_st")
                xv_sb = xpool.tile([128, 8, TQ], BF16, name="xv_st")
                for xs, src in ((xq_sb, xqT), (xk_sb, xkT), (xv_sb, xvT)):
                    nc.sync.dma_start(
                        xs[:],
                        src[:, bass.ts(tcc, TQ)].rearrange("(n p) t -> p n t", p=128),
                    )

                # Q/K projections for this t-chunk: psum[o128, t512], 8 d-tiles
                for xs, w_sb, dst in ((xq_sb, wq_sb, QT), (xk_sb, wk_sb, KT)):
                    for u in range(2):
                        ps = ps_mm.tile([128, TQ], F32, name="ps")
                        for kd in range(8):
                            nc.tensor.matmul(
                                ps[:],
                                w_sb[:, kd, bass.ts(u, 128)],
                                xs[:, kd, :],
                                start=(kd == 0),
                                stop=(kd == 7),
                            )
                        nc.vector.tensor_copy(dst[u][:, bass.ts(tcc, TQ)], ps[:])

                # V projection: natural orientation [t128, feat256] per j-tile
                for jl in range(4):
                    jt = tcc * 4 + jl
                    psv = ps_mm.tile([128, TQ], F32, name="ps")
                    for kd in range(8):
                        nc.tensor.matmul(
                            psv[:, 0:HF],
                            xv_sb[:, kd, bass.ts(jl, 128)],
                            wv_sb[:, kd, :],
                            start=(kd == 0),
                            stop=(kd == 7),
                        )
                    nc.vector.tensor_copy(
                        Vb[:, jt * HPC:(jt + 1) * HPC, 0:DH],
                        psv[:, 0:HF].rearrange("p (h k) -> p h k", k=DH),
                    )

                # Attention for i-chunk ic == tcc, head pairs u=(2u, 2u+1).
                # Scores for both heads of a pair go into one 2-bank PSUM
                # tile (concurrent row-group matmuls, K=64 each), one merged
                # exp per j-tile. Causality handled by free-dim trimming on
                # the diagonal chunk + a 0/1 triangle multiply on the
                # diagonal 128-block (no -1e9 mask add needed).
                ic = tcc
                n_jt = 4 * ic + 4
                skew = 2
                for u in range(2):
                    pctx = [
                        ps_ctx.tile([DH + 1, TQ], F32, name="pctx") for _ in range(2)
                    ]
                    ats = []
                    lows = []
                    for jt in range(n_jt):
                        p = jt - 4 * ic
                        lo = max(p, 0) * 128
                        ps2 = ps_s.tile([128, 2, TQ], F32, name="ps_sc")
                        for h in range(2):
                            nc.tensor.matmul(
                                ps2[:, h, lo:TQ],
                                KT[u][h * DH:(h + 1) * DH, bass.ts(jt, 128)],
                                QT[u][h * DH:(h + 1) * DH, ic * TQ + lo:(ic + 1) * TQ],
                                start=True,
                                stop=True,
                            )
                        at2 = apool.tile([128, 2, TQ], BF16, name="at2")
                        nc.scalar.activation(
                            at2[:, :, lo:TQ], ps2[:, :, lo:TQ], Exp, scale=SCALE
                        )
                        if p >= 0:
                            for h in range(2):
                                nc.vector.tensor_mul(
                                    at2[:, h, lo:lo + 128],
                                    at2[:, h, lo:lo + 128],
                                    tri_sb[:],
                                )
                        ats.append(at2)
                        lows.append(lo)
                        # AV accumulation skewed behind scores for PE/ACT
                        # pipelining
                        if jt >= skew:
                            pv = jt - skew
                            for h in range(2):
                                nc.tensor.matmul(
                                    pctx[h][:, lows[pv]:TQ],
                                    Vb[:, pv * HPC + 2 * u + h, :],
                                    ats[pv][:, h, lows[pv]:TQ],
                                    start=(pv == 0),
                                    stop=False,
                                )
                    for pv in range(max(n_jt - skew, 0), n_jt):
                        for h in range(2):
                            nc.tensor.matmul(
                                pctx[h][:, lows[pv]:TQ],
                                Vb[:, pv * HPC + 2 * u + h, :],
                                ats[pv][:, h, lows[pv]:TQ],
                                start=(pv == 0),
                                stop=(pv == n_jt - 1),
                            )

                    # Normalize: row DH of pctx is the denominator. Broadcast
                    # 1/denom across 64 partitions via a rank-1 f32r matmul.
                    for h in range(2):
                        gh = 2 * u + h
                        rv = rvpool.tile([1, TQ], F32R, name="rvec")
                        with nc.allow_low_precision(reason="f32r 1/denom bcast"):
                            nc.vector.reciprocal(rv[:], pctx[h][DH:DH + 1, :])
                        pb = ps_mm.tile([DH, TQ], F32, name="pb", bufs=1)
                        nc.tensor.matmul(
                            pb[:],
                            ones_sb[:],
                            rv[:],
                            start=True,
                            stop=True,
                        )
                        rb = rbpool.tile([DH, TQ], F32, name="rbt")
                        nc.scalar.copy(rb[:], pb[:])
                        ctxT = cpool.tile([DH, TQ], BF16, name="ctxT")
                        nc.vector.tensor_mul(ctxT[:], pctx[h][0:DH, :], rb[:])
                        nc.sync.dma_start(
                            ccin[ic][gh * DH:(gh + 1) * DH, :], ctxT[:]
                        )

                if sim:
                    for r in range(4):
                        nc.sync.dma_start(
                            agout[ic][r * HF:(r + 1) * HF, :], ccin[ic][:, :]
                        )
                else:
                    nc.gpsimd.collective_compute(
                        "AllGather",
                        mybir.AluOpType.bypass,
                        replica_groups=[[0, 1, 2, 3], [4, 5, 6, 7]],
                        ins=[ccin[ic].opt()],
                        outs=[agout[ic].opt()],
                    )

                # Output projection for this core's query block of chunk ic:
                # stage the [1024 feats, 128 q] slice for group rank r via
                # predicated DMAs (exactly one fires at runtime).
                cst = cstp.tile([128, 8, QB], BF16, name="cst")
                if sim:
                    nc.sync.dma_start(
                        cst[:],
                        agout[ic][:, bass.ts(0, QB)].rearrange(
                            "(n p) q -> p n q", p=128
                        ),
                    )
                else:
                    for r in range(4):
                        nc.sync.dma_start(
                            cst[:],
                            agout[ic][:, bass.ts(r, QB)].rearrange(
                                "(n p) q -> p n q", p=128
                            ),
                            cond=sel_regs[r],
                        )
                for dc in range(2):
                    pso = ps_mm.tile([128, TQ], F32, name="ps")
                    for kt in range(8):
                        nc.tensor.matmul(
                            pso[0:QB, :],
                            cst[:, kt, :],
                            wo_sb[:, kt, bass.ts(dc, TQ)],
                            start=(kt == 0),
                            stop=(kt == 7),
                        )
                    ob = obp.tile([QB, TQ], F32, name="ob")
                    nc.vector.tensor_copy(ob[:], pso[0:QB, :])
                    nc.sync.dma_start(
                        out_q[bass.ts(ic, QB), bass.ts(dc, TQ)], ob[:]
                    )

    nc.finalize()
    return nc


def _make_in_maps(inputs):
    import ml_dtypes

    bf16 = ml_dtypes.bfloat16
    query, key, value = inputs["query"], inputs["key"], inputs["value"]
    Wq, Wk, Wv, Wo = inputs["Wq"], inputs["Wk"], inputs["Wv"], inputs["Wo"]

    tri_blk = np.tril(np.ones((128, 128), np.float32)).T.astype(bf16)
    woT_full = np.ascontiguousarray(np.asarray(Wo, np.float32).T).astype(bf16)

    xT = {}
    for b in range(2):
        xT[("q", b)] = np.ascontiguousarray(np.asarray(query[b], np.float32).T).astype(bf16)
        xT[("k", b)] = np.ascontiguousarray(np.asarray(key[b], np.float32).T).astype(bf16)
        xT[("v", b)] = np.ascontiguousarray(np.asarray(value[b], np.float32).T).astype(bf16)

    in_maps = []
    for c in range(8):
        b, r = divmod(c, 4)
        rs = slice(r * HF, (r + 1) * HF)
        in_maps.append(
            {
                "xqT": xT[("q", b)],
                "xkT": xT[("k", b)],
                "xvT": xT[("v", b)],
                "wqT": np.ascontiguousarray(np.asarray(Wq[rs], np.float32).T).astype(bf16),
                "wkT": np.ascontiguousarray(np.asarray(Wk[rs], np.float32).T).astype(bf16),
                "wvT": np.ascontiguousarray(np.asarray(Wv[rs], np.float32).T).astype(bf16),
                "woT": woT_full,
                "tri": tri_blk,
                "sel": (np.arange(4, dtype=np.uint32) == r).astype(np.uint32)[None, :],
            }
        )
    return in_maps


def _run(inputs, trace=False):
    if "nc" not in _CACHE:
        _CACHE["nc"] = _build_graph()
    nc = _CACHE["nc"]
    in_maps = _make_in_maps(inputs)
    res = run_bass_kernel_spmd(nc, in_maps, core_ids=list(range(8)), trace=trace)

    out = np.empty((B, S, D), np.float32)
    for c in range(8):
        b, r = divmod(c, 4)
        oq = np.asarray(res.results[c]["out_q"])
        for ic in range(4):
            out[b, ic * TQ + r * QB:ic * TQ + (r + 1) * QB, :] = oq[
                ic * QB:(ic + 1) * QB, :
            ]
    return out, res


def kernel(**inputs):
    out, _ = _run(inputs, trace=False)
    return out

